# revision 7
# baseline (speedup 1.0000x reference)
"""AttentiveManifoldMixer Trainium2 kernel (8-core data parallel over batch).

Math: with W3[c,i,j] = conv_w[c*64+i, j], B = conv_b.reshape(C, C),
  s[b]       = sigmoid(fc2 @ relu(fc1 @ mean_hw(x[b])))
  out[b,c,p] = sum_{i,j} W3[c,i,j] * s[b,j] * x[b,i,p] * x[b,j,p]
               + sum_i B[c,i] * x[b,i,p]

The quadratic form is symmetrized over unordered channel pairs grouped by
cyclic diagonal offset d: a feature lane holds x_i * x_j with j-i = d
(mod 64); the per-batch weight (W3[c,i,j]*s_j + W3[c,j,i]*s_i)/mult is
folded on device.  17 chunks x 128 lanes cover d = 0..33 (d=32/33 lanes are
duplicates at higher mult).  This halves the FLOPs of the naive C^2 conv.

v2 dataflow (vs the DRAM-staging baseline):
  * x is cast to bf16 in 4 column quarters; two Act casts write the doubled
    xd = [x; x] directly (no DRAM round trip).  Per-quarter accum feeds the
    SE path early.
  * The 8 rotated variant tiles (A_k = [rot x]x2, B_l = [rot;rot']) are
    built with SBUF->SBUF DMA window copies out of xd (2 per tile, full P).
  * Feature products run on DVE (2x_1P bf16) with the tail chunks of each
    column half offloaded to GPSIMD/Pool so neither engine exceeds the
    PE floor (~31us of matmul).
  * Weight fold: 3 Act muls (t1 = a1*S1) + 6 DVE scalar_tensor_tensor
    (wc = a2*S2 + t1), k-major so chunks 0..2 are ready first.
  * s gathers use single compact-AP DMAs off a doubled s in DRAM.
  * The timing variant wraps the body in For_i_unrolled(max_unroll=4):
    pool-slot rotation + subtile deps pipeline consecutive iterations, so
    the all-engine loop barrier is paid once per 4 iterations.
"""
import sys

sys.path.insert(0, "/opt/trn_rl_repo")

import numpy as np
import ml_dtypes

B, C, H, W = 8, 64, 64, 64
P = H * W                  # 4096 pixels per sample
MID = C // 4
NCHUNK = 17                # feature chunks
NA, NB = 6, 3              # A/B variant tiles; chunk m = 3*(m//3) + m%3
NSUB = 512                 # matmul free-dim subtile
NS = P // NSUB             # psum banks per full sweep
NSPLIT = 2                 # column halves for the TT/GEMM pipeline
HALF = P // NSPLIT
NSH = NS // NSPLIT
NQ = 4                     # cast/accum column quarters
QUART = P // NQ
N_CORES = 8
UNROLL = 4
# chunks whose feature product runs on GPSIMD (per column half)
POOL_M = {(0, 14), (0, 15), (0, 16), (1, 13), (1, 14), (1, 15), (1, 16)}

_CACHE = {}


def _lane_maps():
    """Per-lane (i, j, mult): chunk m = 3k+l, lane q = 64*qhi + qlo:
    i = (qlo - 6k) % 64,  j = (qlo + 2l + qhi) % 64."""
    i_idx = np.zeros((NCHUNK, 128), np.int64)
    j_idx = np.zeros((NCHUNK, 128), np.int64)
    for m in range(NCHUNK):
        k, l = divmod(m, 3)
        for q in range(128):
            qhi, qlo = divmod(q, 64)
            i_idx[m, q] = (qlo - 6 * k) % 64
            j_idx[m, q] = (qlo + 2 * l + qhi) % 64
    lo = np.minimum(i_idx, j_idx)
    hi = np.maximum(i_idx, j_idx)
    key = lo * 64 + hi
    _, inv, counts = np.unique(key, return_inverse=True, return_counts=True)
    mult = counts[inv].reshape(key.shape).astype(np.float32)
    return i_idx, j_idx, mult


def _host_weights(conv_w, fc1_w, fc2_w):
    """Pre-gather conv_w into per-lane arrays a1/a2 of shape (128, 17, 64):
    [lane q, chunk m, out-channel c], bf16."""
    w3 = conv_w.reshape(C, C, C)  # [c, i, j]
    i_idx, j_idx, mult = _lane_maps()
    a1 = np.transpose(w3[:, i_idx, j_idx], (2, 1, 0)) / mult.T[:, :, None]
    a2 = np.transpose(w3[:, j_idx, i_idx], (2, 1, 0)) / mult.T[:, :, None]
    diag = (i_idx == j_idx).T  # [q, m]
    a2[diag] = 0.0
    fc1t = (fc1_w.T / float(P)).copy()   # (64, 16): folds the 1/HW of the mean
    fc2t = fc2_w.T.copy()                # (16, 64)
    return (np.ascontiguousarray(a1, ml_dtypes.bfloat16),
            np.ascontiguousarray(a2, ml_dtypes.bfloat16), fc1t, fc2t)


def _build_program(niter=None):
    """Build the kernel program; with niter, wrap the body in an unrolled
    on-device repeat loop (timing variant)."""
    import concourse.bacc as bacc
    import concourse.bass as bass
    from concourse import mybir
    from concourse.tile import TileContext

    nc = bacc.Bacc("TRN2", target_bir_lowering=False, debug=False)
    dt = mybir.dt

    x_d = nc.dram_tensor("x", [C, P], dt.float32r, kind="ExternalInput")
    a1_d = nc.dram_tensor("a1", [128, NCHUNK, C], dt.bfloat16, kind="ExternalInput")
    a2_d = nc.dram_tensor("a2", [128, NCHUNK, C], dt.bfloat16, kind="ExternalInput")
    f1_d = nc.dram_tensor("fc1t", [C, MID], dt.float32, kind="ExternalInput")
    f2_d = nc.dram_tensor("fc2t", [MID, C], dt.float32, kind="ExternalInput")
    id_d = nc.dram_tensor("ident", [C, C], dt.float32r, kind="ExternalInput")
    out_d = nc.dram_tensor("out", [C, P], dt.float32, kind="ExternalOutput")

    with TileContext(nc) as tc:
        with tc.tile_pool(name="big", bufs=1) as bigp, \
             tc.tile_pool(name="wts", bufs=1) as wtsp, \
             tc.tile_pool(name="sml", bufs=2) as smlp, \
             tc.tile_pool(name="dram", bufs=2, space="DRAM") as dpool, \
             tc.tile_pool(name="feat", bufs=7) as featp, \
             tc.tile_pool(name="outs", bufs=4) as outsp, \
             tc.tile_pool(name="psum", bufs=8, space="PSUM") as psum:

            def xd_win(xd, r0, cols=slice(0, P)):
                """64-row window of xd starting at (possibly odd) row r0."""
                ncols = cols.stop - cols.start
                return bass.AP(tensor=xd.tensor,
                               offset=xd.offset + r0 * P + cols.start,
                               ap=[[P, 64], [1, ncols]])

            def body():
                # ---- resident tiles (bufs=1: reused across iterations,
                # pipelined by subtile-range deps) ----
                xf = bigp.tile([C, P], dt.float32r, name="xf", tag="xf")
                xd = bigp.tile([128, P], dt.bfloat16, name="xd", tag="xd")
                a_t = {k: bigp.tile([128, P], dt.bfloat16, name=f"av{k}",
                                    tag=f"av{k}") for k in range(1, NA)}
                b_t = {l: bigp.tile([128, P], dt.bfloat16, name=f"bv{l}",
                                    tag=f"bv{l}") for l in range(NB)}
                a1s = wtsp.tile([128, NCHUNK, C], dt.bfloat16, name="a1s", tag="a1s")
                a2s = wtsp.tile([128, NCHUNK, C], dt.bfloat16, name="a2s", tag="a2s")
                f1s = wtsp.tile([C, MID], dt.float32, name="f1s", tag="f1s")
                f2s = wtsp.tile([MID, C], dt.float32, name="f2s", tag="f2s")
                ids = wtsp.tile([C, C], dt.float32r, name="ids", tag="ids")
                sums = smlp.tile([C, NQ], dt.float32, name="sums", tag="sums")
                y1 = smlp.tile([MID, 1], dt.float32, name="y1", tag="y1")
                svec = smlp.tile([C, 1], dt.float32, name="svec", tag="svec")
                s1b = smlp.tile([128, NB], dt.float32, name="s1b", tag="s1b")
                s2b = smlp.tile([128, NA], dt.float32, name="s2b", tag="s2b")
                t1 = smlp.tile([128, NCHUNK, C], dt.bfloat16, name="t1", tag="t1")
                wc = smlp.tile([128, NCHUNK, C], dt.bfloat16, name="wc", tag="wc")

                # ---- x load (quarters) + bf16 casts building xd=[x;x];
                # per-quarter accum feeds the SE path early (Act queue) ----
                qsls = [slice(i * QUART, (i + 1) * QUART) for i in range(NQ)]
                nc.scalar.dma_start(out=xf[:, qsls[0]], in_=x_d.ap()[:, qsls[0]])
                nc.scalar.dma_start(out=xf[:, qsls[1]], in_=x_d.ap()[:, qsls[1]])
                nc.scalar.dma_start(out=f1s, in_=f1_d.ap())
                nc.scalar.dma_start(out=f2s, in_=f2_d.ap())
                for q, qsl in enumerate(qsls):
                    if q >= 2:
                        nc.scalar.dma_start(out=xf[:, qsl], in_=x_d.ap()[:, qsl])
                    nc.scalar.activation(xd[0:C, qsl], xf[:, qsl],
                                         mybir.ActivationFunctionType.Copy,
                                         accum_out=sums[:, q:q + 1])
                    nc.scalar.activation(xd[C:128, qsl], xf[:, qsl],
                                         mybir.ActivationFunctionType.Copy)

                # ---- weight loads + variant builds (SP queue).  Each
                # variant tile is 2 full-P window copies out of xd. ----
                nc.sync.dma_start(out=a1s, in_=a1_d.ap())
                nc.sync.dma_start(out=a2s, in_=a2_d.ap())
                nc.sync.dma_start(out=ids, in_=id_d.ap())
                for l in range(NB):
                    nc.sync.dma_start(out=b_t[l][0:C, :], in_=xd_win(xd, 2 * l))
                    nc.sync.dma_start(out=b_t[l][C:128, :],
                                      in_=xd_win(xd, 2 * l + 1))
                for k in range(1, NA):
                    for hrow in range(2):
                        nc.sync.dma_start(out=a_t[k][C * hrow:C * hrow + C, :],
                                          in_=xd_win(xd, C - 6 * k))

                # ---- SE path: s = sigmoid(fc2t.T @ relu(fc1t.T @ sums)) ----
                ps1 = psum.tile([MID, 1], dt.float32, tag="acc", name="ps1")
                for q in range(NQ):
                    nc.tensor.matmul(ps1, f1s, sums[:, q:q + 1], start=(q == 0),
                                     stop=(q == NQ - 1))
                nc.scalar.activation(y1, ps1, mybir.ActivationFunctionType.Relu)
                ps2 = psum.tile([C, 1], dt.float32, tag="acc", name="ps2")
                nc.tensor.matmul(ps2, f2s, y1, start=True, stop=True)
                nc.scalar.activation(svec, ps2,
                                     mybir.ActivationFunctionType.Sigmoid)

                # s -> DRAM twice (s_int = [s; s]); compact-AP gathers:
                # S1b[q, l] = s[(qlo + 2l + qhi) % 64], S2b[q, k] = s[(qlo - 6k) % 64]
                s_int = dpool.tile([2 * C], dt.float32, name="sint", tag="sint")
                nc.scalar.dma_start(out=s_int[0:C][:, None], in_=svec)
                nc.scalar.dma_start(out=s_int[C:2 * C][:, None], in_=svec)
                for qhi in range(2):
                    nc.scalar.dma_start(
                        out=s1b[64 * qhi:64 * qhi + 64, :],
                        in_=bass.AP(tensor=s_int.tensor,
                                    offset=s_int.offset + qhi,
                                    ap=[[1, 64], [2, NB]]))
                for k in range(NA):
                    nc.scalar.dma_start(
                        out=s2b[:, k:k + 1],
                        in_=bass.AP(tensor=s_int.tensor,
                                    offset=s_int.offset + (64 - 6 * k) % 64,
                                    ap=[[0, 2], [1, 64], [0, 1]]))

                # ---- fold s into weights: wc = a1*S1 + a2*S2 (bf16).
                # t1 = a1*S1 on Act (l-strided); wc via 6 DVE stt ops,
                # k-major so wc[:, 0:3] lands first for the GEMM. ----
                for l in range(NB):
                    nc.scalar.mul(t1[:, l::3, :], a1s[:, l::3, :],
                                  s1b[:, l:l + 1])
                for k in range(NA):
                    ms = slice(3 * k, min(3 * k + 3, NCHUNK))
                    nc.vector.scalar_tensor_tensor(
                        wc[:, ms, :], a2s[:, ms, :], s2b[:, k:k + 1],
                        t1[:, ms, :], mybir.AluOpType.mult,
                        mybir.AluOpType.add)

                # ---- main sweep: per column half, 17 feature TTs
                # (DVE + Pool tail) feeding 4 psum banks of GEMM ----
                hsls = [slice(i * HALF, (i + 1) * HALF) for i in range(NSPLIT)]
                for h, hsl in enumerate(hsls):
                    banks = [psum.tile([C, NSUB], dt.float32, tag="acc",
                                       name=f"bank{h}_{j}") for j in range(NSH)]
                    for m in range(NCHUNK):
                        k, l = divmod(m, 3)
                        atile = xd if k == 0 else a_t[k]
                        on_pool = (h, m) in POOL_M
                        # Pool chunks rotate their own slots so their TTs
                        # start as soon as the variant tiles land, instead
                        # of queueing behind the DVE chunks' slot reuse.
                        f = featp.tile([128, HALF], dt.bfloat16,
                                       tag="fp" if on_pool else "f",
                                       bufs=8 if on_pool else None,
                                       name="f")
                        eng = nc.gpsimd if on_pool else nc.vector
                        eng.tensor_mul(f, atile[:, hsl], b_t[l][:, hsl])
                        for j in range(NSH):
                            nc.tensor.matmul(banks[j], wc[:, m, :],
                                             f[:, j * NSUB:(j + 1) * NSUB],
                                             start=(m == 0),
                                             stop=(m == NCHUNK - 1))
                        if m == 5:
                            # conv_b term: += B @ x (float32r, full rate)
                            for j in range(NSH):
                                col = h * HALF + j * NSUB
                                nc.tensor.matmul(banks[j], ids,
                                                 xf[:, col:col + NSUB],
                                                 start=False, stop=False)
                    for j in range(NSH):
                        col = h * HALF + j * NSUB
                        ot = outsp.tile([C, NSUB], dt.float32, tag="o",
                                        name="ot")
                        nc.scalar.copy(ot, banks[j])
                        nc.sync.dma_start(out=out_d.ap()[:, col:col + NSUB],
                                          in_=ot)

            if niter:
                engs = (mybir.EngineType.PE, mybir.EngineType.DVE,
                        mybir.EngineType.SP, mybir.EngineType.Activation,
                        mybir.EngineType.Pool)
                tc.For_i_unrolled_general(
                    start=0, end=niter, step=1,
                    unrollable_body=lambda iv0, unroll: [body() for _ in
                                                         range(unroll)],
                    max_unroll=UNROLL, hint_engines=engs)
            else:
                body()

    nc.compile()
    return nc


def _get_program(niter=None):
    key = ("nc", niter)
    if key not in _CACHE:
        _CACHE[key] = _build_program(niter)
    return _CACHE[key]


def kernel(x, fc1_w, fc2_w, conv_w, conv_b):
    from concourse.bass_utils import run_bass_kernel_spmd

    x = np.asarray(x, np.float32)
    a1, a2, fc1t, fc2t = _host_weights(
        np.asarray(conv_w, np.float32), np.asarray(fc1_w, np.float32),
        np.asarray(fc2_w, np.float32))
    # conv_b contributes sum_i B[c,i]*x_i with B = conv_b.reshape(C, C); the
    # "residual" matmul realizes it with lhsT = B.T (identity-init -> +x).
    ident = np.ascontiguousarray(
        np.asarray(conv_b, np.float32).reshape(C, C).T)
    nc = _get_program()
    in_maps = []
    for b in range(N_CORES):
        in_maps.append({
            "x": np.ascontiguousarray(x[b].reshape(C, P)),
            "a1": a1, "a2": a2, "fc1t": fc1t, "fc2t": fc2t, "ident": ident,
        })
    res = run_bass_kernel_spmd(nc, in_maps, core_ids=list(range(N_CORES)))
    out = np.stack([res.results[b]["out"].reshape(C, H, W)
                    for b in range(N_CORES)], axis=0)
    return out.astype(np.float32)


# revision 9
# speedup vs baseline: 1.0049x; 1.0049x over previous
"""AttentiveManifoldMixer Trainium2 kernel (8-core data parallel over batch).

Math: with W3[c,i,j] = conv_w[c*64+i, j], B = conv_b.reshape(C, C),
  s[b]       = sigmoid(fc2 @ relu(fc1 @ mean_hw(x[b])))
  out[b,c,p] = sum_{i,j} W3[c,i,j] * s[b,j] * x[b,i,p] * x[b,j,p]
               + sum_i B[c,i] * x[b,i,p]

The quadratic form is symmetrized over unordered channel pairs grouped by
cyclic diagonal offset d: a feature lane holds x_i * x_j with j-i = d
(mod 64); the per-batch weight (W3[c,i,j]*s_j + W3[c,j,i]*s_i)/mult is
folded on device.  17 chunks x 128 lanes cover d = 0..33 (d=32/33 lanes are
duplicates at higher mult).  This halves the FLOPs of the naive C^2 conv.

v2 dataflow (vs the DRAM-staging baseline):
  * x is cast to bf16 in 4 column quarters; two Act casts write the doubled
    xd = [x; x] directly (no DRAM round trip).  Per-quarter accum feeds the
    SE path early.
  * The 8 rotated variant tiles (A_k = [rot x]x2, B_l = [rot;rot']) are
    built with SBUF->SBUF DMA window copies out of xd (2 per tile, full P).
  * Feature products run on DVE (2x_1P bf16) with the tail chunks of each
    column half offloaded to GPSIMD/Pool so neither engine exceeds the
    PE floor (~31us of matmul).
  * Weight fold: 3 Act muls (t1 = a1*S1) + 6 DVE scalar_tensor_tensor
    (wc = a2*S2 + t1), k-major so chunks 0..2 are ready first.
  * s gathers use single compact-AP DMAs off a doubled s in DRAM.
  * The timing variant wraps the body in For_i_unrolled(max_unroll=4):
    pool-slot rotation + subtile deps pipeline consecutive iterations, so
    the all-engine loop barrier is paid once per 4 iterations.
"""
import sys

sys.path.insert(0, "/opt/trn_rl_repo")

import numpy as np
import ml_dtypes

B, C, H, W = 8, 64, 64, 64
P = H * W                  # 4096 pixels per sample
MID = C // 4
NCHUNK = 17                # feature chunks
NA, NB = 6, 3              # A/B variant tiles; chunk m = 3*(m//3) + m%3
NSUB = 512                 # matmul free-dim subtile
NS = P // NSUB             # psum banks per full sweep
NSPLIT = 2                 # column halves for the TT/GEMM pipeline
HALF = P // NSPLIT
NSH = NS // NSPLIT
NQ = 4                     # cast/accum column quarters
QUART = P // NQ
N_CORES = 8
UNROLL = 4
# chunks whose feature product runs on GPSIMD (per column half)
POOL_M = {(0, 14), (0, 15), (0, 16), (1, 13), (1, 14), (1, 15), (1, 16)}

_CACHE = {}


def _lane_maps():
    """Per-lane (i, j, mult): chunk m = 3k+l, lane q = 64*qhi + qlo:
    i = (qlo - 6k) % 64,  j = (qlo + 2l + qhi) % 64."""
    i_idx = np.zeros((NCHUNK, 128), np.int64)
    j_idx = np.zeros((NCHUNK, 128), np.int64)
    for m in range(NCHUNK):
        k, l = divmod(m, 3)
        for q in range(128):
            qhi, qlo = divmod(q, 64)
            i_idx[m, q] = (qlo - 6 * k) % 64
            j_idx[m, q] = (qlo + 2 * l + qhi) % 64
    lo = np.minimum(i_idx, j_idx)
    hi = np.maximum(i_idx, j_idx)
    key = lo * 64 + hi
    _, inv, counts = np.unique(key, return_inverse=True, return_counts=True)
    mult = counts[inv].reshape(key.shape).astype(np.float32)
    return i_idx, j_idx, mult


def _host_weights(conv_w, fc1_w, fc2_w):
    """Pre-gather conv_w into per-lane arrays a1/a2 of shape (128, 17, 64):
    [lane q, chunk m, out-channel c], bf16."""
    w3 = conv_w.reshape(C, C, C)  # [c, i, j]
    i_idx, j_idx, mult = _lane_maps()
    a1 = np.transpose(w3[:, i_idx, j_idx], (2, 1, 0)) / mult.T[:, :, None]
    a2 = np.transpose(w3[:, j_idx, i_idx], (2, 1, 0)) / mult.T[:, :, None]
    diag = (i_idx == j_idx).T  # [q, m]
    a2[diag] = 0.0
    fc1t = (fc1_w.T / float(P)).copy()   # (64, 16): folds the 1/HW of the mean
    fc2t = fc2_w.T.copy()                # (16, 64)
    return (np.ascontiguousarray(a1, ml_dtypes.bfloat16),
            np.ascontiguousarray(a2, ml_dtypes.bfloat16), fc1t, fc2t)


def _build_program(niter=None):
    """Build the kernel program; with niter, wrap the body in an unrolled
    on-device repeat loop (timing variant)."""
    import concourse.bacc as bacc
    import concourse.bass as bass
    from concourse import mybir
    from concourse.tile import TileContext

    nc = bacc.Bacc("TRN2", target_bir_lowering=False, debug=False)
    dt = mybir.dt

    x_d = nc.dram_tensor("x", [C, P], dt.float32r, kind="ExternalInput")
    a1_d = nc.dram_tensor("a1", [128, NCHUNK, C], dt.bfloat16, kind="ExternalInput")
    a2_d = nc.dram_tensor("a2", [128, NCHUNK, C], dt.bfloat16, kind="ExternalInput")
    f1_d = nc.dram_tensor("fc1t", [C, MID], dt.float32, kind="ExternalInput")
    f2_d = nc.dram_tensor("fc2t", [MID, C], dt.float32, kind="ExternalInput")
    id_d = nc.dram_tensor("ident", [C, C], dt.float32r, kind="ExternalInput")
    out_d = nc.dram_tensor("out", [C, P], dt.float32, kind="ExternalOutput")

    with TileContext(nc) as tc:
        with tc.tile_pool(name="big", bufs=1) as bigp, \
             tc.tile_pool(name="wts", bufs=1) as wtsp, \
             tc.tile_pool(name="sml", bufs=2) as smlp, \
             tc.tile_pool(name="dram", bufs=2, space="DRAM") as dpool, \
             tc.tile_pool(name="feat", bufs=7) as featp, \
             tc.tile_pool(name="outs", bufs=4) as outsp, \
             tc.tile_pool(name="psum", bufs=8, space="PSUM") as psum:

            def xd_win(xd, r0, cols=slice(0, P)):
                """64-row window of xd starting at (possibly odd) row r0."""
                ncols = cols.stop - cols.start
                return bass.AP(tensor=xd.tensor,
                               offset=xd.offset + r0 * P + cols.start,
                               ap=[[P, 64], [1, ncols]])

            def body():
                # ---- resident tiles (bufs=1: reused across iterations,
                # pipelined by subtile-range deps) ----
                xf = bigp.tile([C, P], dt.float32r, name="xf", tag="xf",
                               bufs=2)
                xd = bigp.tile([128, P], dt.bfloat16, name="xd", tag="xd")
                a_t = {k: bigp.tile([128, P], dt.bfloat16, name=f"av{k}",
                                    tag=f"av{k}") for k in range(NA)}
                b_t = {l: bigp.tile([128, P], dt.bfloat16, name=f"bv{l}",
                                    tag=f"bv{l}") for l in range(NB)}
                a1s = wtsp.tile([128, NCHUNK, C], dt.bfloat16, name="a1s", tag="a1s")
                a2s = wtsp.tile([128, NCHUNK, C], dt.bfloat16, name="a2s", tag="a2s")
                f1s = wtsp.tile([C, MID], dt.float32, name="f1s", tag="f1s")
                f2s = wtsp.tile([MID, C], dt.float32, name="f2s", tag="f2s")
                ids = wtsp.tile([C, C], dt.float32r, name="ids", tag="ids")
                sums = smlp.tile([C, NQ], dt.float32, name="sums", tag="sums")
                y1 = smlp.tile([MID, 1], dt.float32, name="y1", tag="y1")
                svec = smlp.tile([C, 1], dt.float32, name="svec", tag="svec")
                s1b = smlp.tile([128, NB], dt.float32, name="s1b", tag="s1b")
                s2b = smlp.tile([128, NA], dt.float32, name="s2b", tag="s2b")
                t1 = smlp.tile([128, NCHUNK, C], dt.bfloat16, name="t1", tag="t1")
                wc = smlp.tile([128, NCHUNK, C], dt.bfloat16, name="wc", tag="wc")

                # ---- x load (quarters) + bf16 casts building xd=[x;x];
                # per-quarter accum feeds the SE path early (Act queue) ----
                qsls = [slice(i * QUART, (i + 1) * QUART) for i in range(NQ)]
                nc.scalar.dma_start(out=xf[:, qsls[0]], in_=x_d.ap()[:, qsls[0]])
                nc.scalar.dma_start(out=xf[:, qsls[1]], in_=x_d.ap()[:, qsls[1]])
                nc.scalar.dma_start(out=f1s, in_=f1_d.ap())
                nc.scalar.dma_start(out=f2s, in_=f2_d.ap())
                for q, qsl in enumerate(qsls):
                    if q >= 2:
                        nc.scalar.dma_start(out=xf[:, qsl], in_=x_d.ap()[:, qsl])
                    nc.scalar.activation(xd[0:C, qsl], xf[:, qsl],
                                         mybir.ActivationFunctionType.Copy,
                                         accum_out=sums[:, q:q + 1])
                    nc.scalar.activation(xd[C:128, qsl], xf[:, qsl],
                                         mybir.ActivationFunctionType.Copy)

                # ---- weight loads + variant builds (SP queue).  Each
                # variant tile is 2 full-P window copies out of xd. ----
                nc.sync.dma_start(out=a1s, in_=a1_d.ap())
                nc.sync.dma_start(out=a2s, in_=a2_d.ap())
                nc.sync.dma_start(out=ids, in_=id_d.ap())
                for l in range(NB):
                    nc.sync.dma_start(out=b_t[l][0:C, :], in_=xd_win(xd, 2 * l))
                    nc.sync.dma_start(out=b_t[l][C:128, :],
                                      in_=xd_win(xd, 2 * l + 1))
                for k in range(NA):
                    for hrow in range(2):
                        nc.sync.dma_start(out=a_t[k][C * hrow:C * hrow + C, :],
                                          in_=xd_win(xd, (C - 6 * k) % C))

                # ---- SE path: s = sigmoid(fc2t.T @ relu(fc1t.T @ sums)) ----
                ps1 = psum.tile([MID, 1], dt.float32, tag="acc", name="ps1")
                for q in range(NQ):
                    nc.tensor.matmul(ps1, f1s, sums[:, q:q + 1], start=(q == 0),
                                     stop=(q == NQ - 1))
                nc.scalar.activation(y1, ps1, mybir.ActivationFunctionType.Relu)
                ps2 = psum.tile([C, 1], dt.float32, tag="acc", name="ps2")
                nc.tensor.matmul(ps2, f2s, y1, start=True, stop=True)
                nc.scalar.activation(svec, ps2,
                                     mybir.ActivationFunctionType.Sigmoid)

                # s -> DRAM twice (s_int = [s; s]); compact-AP gathers:
                # S1b[q, l] = s[(qlo + 2l + qhi) % 64], S2b[q, k] = s[(qlo - 6k) % 64]
                s_int = dpool.tile([2 * C], dt.float32, name="sint", tag="sint")
                nc.scalar.dma_start(out=s_int[0:C][:, None], in_=svec)
                nc.scalar.dma_start(out=s_int[C:2 * C][:, None], in_=svec)
                for qhi in range(2):
                    nc.scalar.dma_start(
                        out=s1b[64 * qhi:64 * qhi + 64, :],
                        in_=bass.AP(tensor=s_int.tensor,
                                    offset=s_int.offset + qhi,
                                    ap=[[1, 64], [2, NB]]))
                for k in range(NA):
                    nc.gpsimd.dma_start(
                        out=s2b[:, k:k + 1],
                        in_=bass.AP(tensor=s_int.tensor,
                                    offset=s_int.offset + (64 - 6 * k) % 64,
                                    ap=[[0, 2], [1, 64], [0, 1]]))

                # ---- fold s into weights: wc = a1*S1 + a2*S2 (bf16).
                # t1 = a1*S1 on Act (l-strided); wc via 6 DVE stt ops,
                # k-major so wc[:, 0:3] lands first for the GEMM. ----
                for l in range(NB):
                    nc.scalar.mul(t1[:, l::3, :], a1s[:, l::3, :],
                                  s1b[:, l:l + 1])
                for k in range(NA):
                    ms = slice(3 * k, min(3 * k + 3, NCHUNK))
                    nc.vector.scalar_tensor_tensor(
                        wc[:, ms, :], a2s[:, ms, :], s2b[:, k:k + 1],
                        t1[:, ms, :], mybir.AluOpType.mult,
                        mybir.AluOpType.add)

                # ---- main sweep: per column half, 17 feature TTs
                # (DVE + Pool tail) feeding 4 psum banks of GEMM ----
                hsls = [slice(i * HALF, (i + 1) * HALF) for i in range(NSPLIT)]
                for h, hsl in enumerate(hsls):
                    banks = [psum.tile([C, NSUB], dt.float32, tag="acc",
                                       name=f"bank{h}_{j}") for j in range(NSH)]
                    for m in range(NCHUNK):
                        k, l = divmod(m, 3)
                        atile = a_t[k]
                        on_pool = (h, m) in POOL_M
                        # Pool chunks rotate their own slots so their TTs
                        # start as soon as the variant tiles land, instead
                        # of queueing behind the DVE chunks' slot reuse.
                        f = featp.tile([128, HALF], dt.bfloat16,
                                       tag="fp" if on_pool else "f",
                                       bufs=7 if on_pool else 9,
                                       name="f")
                        eng = nc.gpsimd if on_pool else nc.vector
                        eng.tensor_mul(f, atile[:, hsl], b_t[l][:, hsl])
                        for j in range(NSH):
                            nc.tensor.matmul(banks[j], wc[:, m, :],
                                             f[:, j * NSUB:(j + 1) * NSUB],
                                             start=(m == 0),
                                             stop=(m == NCHUNK - 1))
                        if m == 5:
                            # conv_b term: += B @ x (float32r, full rate)
                            for j in range(NSH):
                                col = h * HALF + j * NSUB
                                nc.tensor.matmul(banks[j], ids,
                                                 xf[:, col:col + NSUB],
                                                 start=False, stop=False)
                    for j in range(NSH):
                        col = h * HALF + j * NSUB
                        ot = outsp.tile([C, NSUB], dt.float32, tag="o",
                                        name="ot")
                        nc.scalar.copy(ot, banks[j])
                        nc.sync.dma_start(out=out_d.ap()[:, col:col + NSUB],
                                          in_=ot)

            if niter:
                engs = (mybir.EngineType.PE, mybir.EngineType.DVE,
                        mybir.EngineType.SP, mybir.EngineType.Activation,
                        mybir.EngineType.Pool)
                tc.For_i_unrolled_general(
                    start=0, end=niter, step=1,
                    unrollable_body=lambda iv0, unroll: [body() for _ in
                                                         range(unroll)],
                    max_unroll=UNROLL, hint_engines=engs)
            else:
                body()

    nc.compile()
    return nc


def _get_program(niter=None):
    key = ("nc", niter)
    if key not in _CACHE:
        _CACHE[key] = _build_program(niter)
    return _CACHE[key]


def kernel(x, fc1_w, fc2_w, conv_w, conv_b):
    from concourse.bass_utils import run_bass_kernel_spmd

    x = np.asarray(x, np.float32)
    a1, a2, fc1t, fc2t = _host_weights(
        np.asarray(conv_w, np.float32), np.asarray(fc1_w, np.float32),
        np.asarray(fc2_w, np.float32))
    # conv_b contributes sum_i B[c,i]*x_i with B = conv_b.reshape(C, C); the
    # "residual" matmul realizes it with lhsT = B.T (identity-init -> +x).
    ident = np.ascontiguousarray(
        np.asarray(conv_b, np.float32).reshape(C, C).T)
    nc = _get_program()
    in_maps = []
    for b in range(N_CORES):
        in_maps.append({
            "x": np.ascontiguousarray(x[b].reshape(C, P)),
            "a1": a1, "a2": a2, "fc1t": fc1t, "fc2t": fc2t, "ident": ident,
        })
    res = run_bass_kernel_spmd(nc, in_maps, core_ids=list(range(N_CORES)))
    out = np.stack([res.results[b]["out"].reshape(C, H, W)
                    for b in range(N_CORES)], axis=0)
    return out.astype(np.float32)
